# revision 19
# baseline (speedup 1.0000x reference)
"""Trainium2 Bass kernel v3 for nn_Engram (hashed n-gram embedding + ShortConv gate).

Self-contained: hardcodes all shapes. kernel(**inputs) -> full output [4,2048,4,1024].

Sharding: 8 cores; core c handles batch b=c//2, token half h=c%2 (1024 tokens).
Embedding table replicated per core, quad-packed (4 rows / 512B unit) per head so
int16 dma_gather indices cover a head's vocab. Table pre-scaled x64 so fp8 copies
of the embeddings are well-ranged (value_w pre-scaled 1/64 to compensate).

v3 changes vs v2 (426us baseline):
- keys einsum in fp8e4m3 DoubleRow perf mode (256-deep contraction, 2.1x PE
  throughput measured). Only the gate path sees the fp8 error; value path bf16.
- conv-before-keys per (hf,g) so kps PSUM is consumed immediately; k_sb (64*keys
  + 64*kb, bf16) written once by Act and fans out to k2 (DVE), m (Pool stt).
- elementwise work split across DVE / Act / Pool so all stay under PE's shadow.
- merged-half gathers (16 calls of 1024 idxs), head-pair select buffers feeding
  [128,128] PE transposes in 4-block PSUM groups, single-op copies to bf16
  (value path) and fp8-pair (keys path) layouts.
- weights (kw8 fp8, diag, vw) streamed per (hf,g) through small pools.
"""
import sys
sys.path.insert(0, "/opt/trn_rl_repo")
from contextlib import ExitStack

import numpy as np
import ml_dtypes

import concourse.bass as bass
import concourse.mybir as mybir
import concourse.tile as tile
from concourse import bacc
from concourse.bass_utils import run_bass_kernel_spmd

BF16 = ml_dtypes.bfloat16
F8NP = ml_dtypes.float8_e4m3fn if hasattr(ml_dtypes, "float8_e4m3fn") else ml_dtypes.float8_e4m3
F32 = mybir.dt.float32
BF = mybir.dt.bfloat16
F8 = mybir.dt.float8e4
I16 = mybir.dt.int16
MULT = mybir.AluOpType.mult
ADD = mybir.AluOpType.add
AF = mybir.ActivationFunctionType
DR = mybir.MatmulPerfMode.DoubleRow

B, T, G, HID = 4, 2048, 4, 1024
NH, D = 16, 64
C = G * HID
KS, DIL = 4, 3
HALO = (KS - 1) * DIL  # 9
W = 1024 + HALO        # 1033
EPS_SC, EPS_RMS = 1e-5, 1e-6
VOCAB_PER_NGRAM = (129280, 129280)
N_CORES = 8
TOK = (B * T) // N_CORES  # 1024
NCH = 32
HT = 512               # tokens per hf half
NQ = 4                 # swdge queues
ESC = 64.0             # emb table pre-scale (exact power of 2)
import os as _os
SIM_SILU = _os.environ.get("SIM_SILU", "0") == "1"


def _isprime(n):
    if n < 2:
        return False
    if n % 2 == 0:
        return n == 2
    i = 3
    while i * i <= n:
        if n % i == 0:
            return False
        i += 2
    return True


def _head_vocab_sizes():
    seen, out = set(), []
    for v in VOCAB_PER_NGRAM:
        start = v - 1
        for _ in range(NH // 2):
            c = start + 1
            while (not _isprime(c)) or (c in seen):
                c += 1
            seen.add(c)
            out.append(c)
            start = c
    return out


LIST_OF_N = _head_vocab_sizes()
OFFSETS = np.concatenate([[0], np.cumsum(LIST_OF_N[:-1])]).astype(np.int64)
NUNITS = int(max((n + 3) // 4 for n in LIST_OF_N))

_CACHE = {}

# front-phase psum free-dim chunking (2048B per bank -> max 512 f32 columns)
CHUNKS = [(0, 259), (259, 259), (518, 259), (777, 256)]


def _build_bass(reps=1, hw_loop=False, phases=(1, 2, 3)):
    nc = bacc.Bacc("TRN2", target_bir_lowering=False, num_swdge_queues=NQ)

    hT_d = nc.dram_tensor("hT", [128, NCH, W], BF, kind="ExternalInput")
    qtab_d = nc.dram_tensor("qtab", [NH, NUNITS, 256], BF, kind="ExternalInput")
    idx_d = nc.dram_tensor("idx16", [128, 2, NH, HT // 16], I16, kind="ExternalInput")
    mask_d = nc.dram_tensor("mask", [128, 2, NH, 4, 4], BF, kind="ExternalInput")
    vwT_d = nc.dram_tensor("vwT", [8, 128, 8, 128], BF, kind="ExternalInput")
    kw8_d = nc.dram_tensor("kw8", [G, 128, 8, 4, 2, 128], F8, kind="ExternalInput")
    diag_d = nc.dram_tensor("diag", [G, 128, 8, KS, 128], BF, kind="ExternalInput")
    w12_d = nc.dram_tensor("w12pad", [128, G, 8, 4], BF, kind="ExternalInput")
    ones_d = nc.dram_tensor("onespad", [128, G, 4], BF, kind="ExternalInput")
    oh_d = nc.dram_tensor("onehot", [G, 4, 128], BF, kind="ExternalInput")
    ident_d = nc.dram_tensor("ident", [128, 128], BF, kind="ExternalInput")
    kb_d = nc.dram_tensor("kb64", [128, G, 8], F32, kind="ExternalInput")
    vb_d = nc.dram_tensor("vb", [128, 8], F32, kind="ExternalInput")
    outT_d = nc.dram_tensor("outT", [G, 8, 128, TOK], BF, kind="ExternalOutput")

    with ExitStack() as ctx:
        tc = ctx.enter_context(tile.TileContext(nc))
        const = ctx.enter_context(tc.tile_pool(name="const", bufs=1))
        p_xb = ctx.enter_context(tc.tile_pool(name="xb", bufs=1))
        p_x2 = ctx.enter_context(tc.tile_pool(name="x2", bufs=1))
        p_rbc = ctx.enter_context(tc.tile_pool(name="rbc", bufs=2))
        p_nrm = ctx.enter_context(tc.tile_pool(name="nrm", bufs=1))
        p_row = ctx.enter_context(tc.tile_pool(name="row", bufs=3))
        p_ga = ctx.enter_context(tc.tile_pool(name="ga", bufs=2))
        p_eba = ctx.enter_context(tc.tile_pool(name="eba", bufs=4))
        p_emb = ctx.enter_context(tc.tile_pool(name="emb", bufs=1))
        p_emb8 = ctx.enter_context(tc.tile_pool(name="emb8", bufs=1))
        p_kw = ctx.enter_context(tc.tile_pool(name="kw", bufs=2))
        p_dg = ctx.enter_context(tc.tile_pool(name="dg", bufs=2))
        p_cv = ctx.enter_context(tc.tile_pool(name="cv", bufs=1))
        p_ks = ctx.enter_context(tc.tile_pool(name="ks", bufs=1))
        p_c2 = ctx.enter_context(tc.tile_pool(name="c2", bufs=1))
        p_k2 = ctx.enter_context(tc.tile_pool(name="k2", bufs=1))
        p_m = ctx.enter_context(tc.tile_pool(name="m", bufs=1))
        p_vw = ctx.enter_context(tc.tile_pool(name="vw", bufs=2))
        p_vs = ctx.enter_context(tc.tile_pool(name="vs", bufs=2))
        p_gb = ctx.enter_context(tc.tile_pool(name="gb", bufs=1))
        p_ob = ctx.enter_context(tc.tile_pool(name="ob", bufs=2))

        # ---- constants ----
        def cdma(dst, src):
            nc.sync.dma_start(dst, src)
        w12_t = const.tile([128, G, 8, 4], BF)
        cdma(w12_t[:], w12_d[:])
        ones_t = const.tile([128, G, 4], BF)
        cdma(ones_t[:], ones_d[:])
        oh_t = const.tile([4, G, 128], BF)
        cdma(oh_t[:], oh_d[:].rearrange("g k m -> k g m"))
        id_t = const.tile([128, 128], BF)
        cdma(id_t[:], ident_d[:])
        kb_t = const.tile([128, G, 8], F32)
        cdma(kb_t[:], kb_d[:])
        vb_t = const.tile([128, 8], F32)
        cdma(vb_t[:], vb_d[:])
        mask_t = const.tile([128, 2, NH, 4, 4], BF)
        cdma(mask_t[:], mask_d[:])
        idx_t = const.tile([128, 2, NH, HT // 16], I16)
        cdma(idx_t[:], idx_d[:])
        eps_sc_t = const.tile([4, 1], F32)
        nc.vector.memset(eps_sc_t[:], EPS_SC)
        eps_rms_t = const.tile([4, 1], F32)
        nc.vector.memset(eps_rms_t[:], EPS_RMS)
        one_sc = const.tile([128, 1], F32)
        nc.vector.memset(one_sc[:], 1.0)

        xb_t = p_xb.tile([128, NCH, W], BF)
        embsT = p_emb.tile([128, 8, TOK], BF)
        embsF8 = p_emb8.tile([128, 4, 2, TOK], F8)

        if hw_loop:
            _loop_cm = tc.For_i(0, reps, 1)
        else:
            _loop_cm = None

        for _rep in range(1 if hw_loop else reps):
            if _loop_cm is not None:
                _loop_cm.__enter__()
            helpers = {}

            # ---- gathers: per (half, head), spread over queues ----
            ga_tiles = {}

            def emit_gather(hf, j):
                ga = p_ga.tile([128, 4, 256], BF, tag="ga")
                nc.gpsimd.dma_gather(
                    ga[:], qtab_d[j, :, :], idx_t[:, hf, j, :], HT, HT, 256,
                    queue_num=(hf * NH + j) % NQ)
                ga_tiles[(hf, j)] = ga

            # ---- select: interleaved-quad table layout [d, s]; mask-mult on
            # the DVE 2x path (mask broadcast over d via permuted AP), two
            # s-folds; head pair (2j, 2j+1) fills eba [128, 4blk, 2, 64].
            def emit_select(hf, j, eba):
                ga = ga_tiles.pop((hf, j))
                ga5 = ga[:].rearrange("p b (d s) -> p b d s", s=4)
                mb = (mask_t[:, hf, j, :, :].to_broadcast((128, 4, 4, D))
                      .rearrange("p b s d -> p b d s"))
                nc.vector.tensor_tensor(out=ga5, in0=ga5, in1=mb, op=MULT)
                nc.vector.tensor_tensor(
                    out=ga5[:, :, :, 0:2], in0=ga5[:, :, :, 0:2],
                    in1=ga5[:, :, :, 2:4], op=ADD)
                nc.vector.tensor_tensor(
                    out=eba[:, :, j % 2, :], in0=ga5[:, :, :, 0:1],
                    in1=ga5[:, :, :, 1:2], op=ADD)

            # first gathers go out before any bulk DMA so selects start early
            if 2 in phases:
                for j in range(4):
                    emit_gather(0, j)

            def emit_hT_chunk(cq):
                if 1 in phases:
                    nc.sync.dma_start(xb_t[:, cq * 8 : (cq + 1) * 8, :],
                                      hT_d[:, cq * 8 : (cq + 1) * 8, :])

            with (
                tc.tile_pool(name="psf", bufs=1, space="PSUM") as psf,
                tc.tile_pool(name="pst", bufs=2, space="PSUM") as pst_pool,
            ):
                ss_cs = None
                if 1 in phases:
                    ss_cs = [psf.tile([4, 512], F32, tag=f"ssc{i}", name=f"ssc{i}")
                             for i in range(len(CHUNKS))]

                def emit_front_g(g):
                    # squares on Act (frees DVE for selects), ones-reduce on PE
                    for ci, (c0, cl) in enumerate(CHUNKS):
                        x2 = p_x2.tile([128, 8, 272], BF, tag="x2")
                        nc.scalar.activation(
                            x2[:, :, :cl], xb_t[:, g * 8 : (g + 1) * 8, c0 : c0 + cl],
                            AF.Square)
                        for o in range(8):
                            nc.tensor.matmul(
                                ss_cs[ci][:, :cl], ones_t[:, g, :], x2[:, o, :cl],
                                start=(g == 0 and o == 0), stop=(g == 3 and o == 7))

                def emit_transpose_pair(pool, hf, jp, eba):
                    pst = pool.tile([128, 4, 128], BF, tag="pst")
                    for blk in range(4):
                        nc.tensor.transpose(
                            pst[:, blk, :],
                            eba[:, blk, :, :].rearrange("p a b -> p (a b)"),
                            id_t[:])
                    col = hf * HT
                    nc.vector.tensor_copy(
                        embsT[:, jp, col : col + HT],
                        pst[:].rearrange("p b q -> p (b q)"))
                    nc.scalar.activation(
                        embsF8[:, jp // 2, jp % 2, col : col + HT],
                        pst[:].rearrange("p b q -> p (b q)"), AF.Copy)

                def emit_half_select(hf):
                    for jp in range(8):
                        eba = p_eba.tile([128, 4, 2, D], BF, tag="eba")
                        emit_select(hf, 2 * jp, eba)
                        nj = 2 * jp + 2
                        if nj < NH:
                            emit_gather(hf, nj)
                        elif hf == 0:
                            emit_gather(1, nj - NH)
                        emit_select(hf, 2 * jp + 1, eba)
                        nj = 2 * jp + 3
                        if nj < NH:
                            emit_gather(hf, nj)
                        elif hf == 0:
                            emit_gather(1, nj - NH)
                        emit_transpose_pair(pst_pool, hf, jp, eba)
                        if hf == 0 and jp % 2 == 1:
                            emit_hT_chunk(jp // 2)
                            if jp == 7 and 2 in phases:
                                # prefetch g0 keys weights ahead of the main loop
                                for oq in range(2):
                                    kwp = p_kw.tile([128, 2, 4, 2, 128], F8, tag="kw")
                                    nc.sync.dma_start(
                                        kwp[:], kw8_d[0, :, 2 * oq : 2 * oq + 2, :, :, :])
                                    helpers.setdefault("kwpre", []).append(kwp)
                            if 1 in phases:
                                emit_front_g(jp // 2)

                if 2 in phases:
                    emit_half_select(0)
                elif 1 in phases:
                    for g in range(G):
                        emit_hT_chunk(g)
                        emit_front_g(g)
                helpers["select"] = emit_select
                helpers["gather"] = emit_gather
                helpers["transpose"] = emit_transpose_pair

                # ---- rsqrt + per-group broadcast + in-place normalize ----
                if 1 in phases:
                    rsb = p_nrm.tile([4, W], F32, tag="rsb", name="rsb")
                    for ci, (c0, cl) in enumerate(CHUNKS):
                        nc.scalar.activation(rsb[:, c0 : c0 + cl], ss_cs[ci][:, :cl],
                                             AF.Sqrt, bias=eps_sc_t[:], scale=1.0 / HID)
                    nc.vector.reciprocal(rsb[:], rsb[:])
                    rsb_bf = p_nrm.tile([4, W], BF, tag="rsbbf", name="rsbbf")
                    nc.scalar.activation(rsb_bf[:], rsb[:], AF.Copy)
                    for g in range(G):
                        rbcs = p_rbc.tile([128, W], BF, tag="rbcs")
                        for c0, cl in CHUNKS:
                            rb_ps = pst_pool.tile([128, 512], F32, tag="rbps")
                            nc.tensor.matmul(rb_ps[:, :cl], oh_t[:, g, :],
                                             rsb_bf[:, c0 : c0 + cl], start=True, stop=True)
                            nc.scalar.activation(rbcs[:, c0 : c0 + cl], rb_ps[:, :cl], AF.Copy)
                        nc.vector.tensor_tensor(
                            out=xb_t[:, g * 8 : (g + 1) * 8, :],
                            in0=xb_t[:, g * 8 : (g + 1) * 8, :],
                            in1=rbcs[:].rearrange("p (o t) -> p o t", o=1).to_broadcast((128, 8, W)),
                            op=MULT)
            # =============== main per-half pipeline ===============
            # hf1 selects/transposes are woven into hf0's g-loop so the DVE
            # select stream and PE transpose slots never block the hf0 path.
            with (
                tc.tile_pool(name="ps3k", bufs=3, space="PSUM") as ps3k,
                tc.tile_pool(name="ps3r", bufs=2, space="PSUM") as ps3r,
                tc.tile_pool(name="pstm", bufs=2, space="PSUM") as pst_main,
            ):
                eba_hf1 = {}

                def emit_sel_pair_hf1(jp):
                    eba = p_eba.tile([128, 4, 2, D], BF, tag="eba")
                    for jj in range(2):
                        j = 2 * jp + jj
                        helpers["select"](1, j, eba)
                        if j + 2 < NH:
                            helpers["gather"](1, j + 2)
                    eba_hf1[jp] = eba

                def value_half(hf):
                    vsb = p_vs.tile([128, 8, HT], BF, tag="vsb", name=f"vsb{hf}")
                    for dt in range(8):
                        vw = p_vw.tile([128, 8, 128], BF, tag="vw")
                        nc.sync.dma_start(vw[:], vwT_d[dt, :, :, :])
                        vps = ps3k.tile([128, HT], F32, tag="kps")
                        for e in range(8):
                            nc.tensor.matmul(
                                vps[:], vw[:, e, :],
                                embsT[:, e, hf * HT : (hf + 1) * HT],
                                start=(e == 0), stop=(e == 7))
                        nc.scalar.activation(vsb[:, dt, :], vps[:], AF.Identity,
                                             bias=vb_t[:, dt : dt + 1])
                    return vsb

                for hf in range(2):
                    if 3 not in phases:
                        break
                    acc_ps = ps3r.tile([68, HT], F32, tag="acc", name=f"acc{hf}")
                    vsb = None
                    pend = None

                    def emit_acc(g, cvg, k_sb):
                        # c2/k2 on DVE, m on Pool; 24 reduce matmuls on PE
                        first = g == 0
                        last = g == 3
                        for oh in range(4):
                            sl = slice(2 * oh, 2 * oh + 2)
                            c2 = p_c2.tile([128, 2, HT], BF, tag="c2")
                            nc.vector.tensor_tensor(out=c2[:], in0=cvg[:, sl, :],
                                                    in1=cvg[:, sl, :], op=MULT)
                            k2 = p_k2.tile([128, 2, HT], BF, tag="k2")
                            nc.vector.tensor_tensor(out=k2[:], in0=k_sb[:, sl, :],
                                                    in1=k_sb[:, sl, :], op=MULT)
                            m = p_m.tile([128, 2, HT], BF, tag="m")
                            nc.vector.tensor_tensor(
                                out=m[:], in0=k_sb[:, sl, :], in1=cvg[:, sl, :],
                                op=MULT)
                            for o4 in range(2):
                                o = 2 * oh + o4
                                st = first and o == 0
                                sp = last and o == 7
                                nc.tensor.matmul(acc_ps[0:4, :], ones_t[:, g, :],
                                                 c2[:, o4, :], start=st, stop=sp)
                                nc.tensor.matmul(acc_ps[32:36, :], ones_t[:, g, :],
                                                 k2[:, o4, :], start=st, stop=sp)
                                nc.tensor.matmul(acc_ps[64:68, :], w12_t[:, g, o, :],
                                                 m[:, o4, :], start=st, stop=sp)

                    for g in range(G):
                        # ---- keys for (hf, g): fp8 DoubleRow + k_sb ----
                        k_sb = None
                        if 2 in phases:
                            k_sb = p_ks.tile([128, 8, HT], BF, tag="ksb")
                            for o in range(8):
                                if o % 2 == 0:
                                    pre = helpers.get("kwpre") if (hf == 0 and g == 0 and o < 4) else None
                                    if pre:
                                        kw = pre.pop(0)
                                    else:
                                        kw = p_kw.tile([128, 2, 4, 2, 128], F8, tag="kw")
                                        nc.sync.dma_start(kw[:], kw8_d[g, :, o : o + 2, :, :, :])
                                kps = ps3k.tile([128, HT], F32, tag="kps")
                                for ep in range(4):
                                    nc.tensor.matmul(
                                        kps[:], kw[:, o % 2, ep, :, :],
                                        embsF8[:, ep, :, hf * HT : (hf + 1) * HT],
                                        start=(ep == 0), stop=(ep == 3),
                                        perf_mode=DR)
                                nc.scalar.activation(k_sb[:, o, :], kps[:], AF.Identity,
                                                     bias=kb_t[:, g, o : o + 1], scale=1.0 / ESC)

                        # ---- conv for (hf, g): diag matmuls + silu ----
                        cvg = None
                        if 1 in phases:
                            cvg = p_cv.tile([128, 8, HT], BF, tag="cvg")
                            for ci8 in range(8):
                                if ci8 % 4 == 0:
                                    dg = p_dg.tile([128, 4, KS, 128], BF, tag="dg")
                                    nc.sync.dma_start(
                                        dg[:], diag_d[g, :, ci8 : ci8 + 4, :, :])
                                cps = ps3k.tile([128, HT], F32, tag="kps")
                                for j in range(KS):
                                    lo = hf * HT + 3 * j
                                    nc.tensor.matmul(
                                        cps[:], dg[:, ci8 % 4, j, :],
                                        xb_t[:, g * 8 + ci8, lo : lo + HT],
                                        start=(j == 0), stop=(j == KS - 1))
                                nc.scalar.activation(cvg[:, ci8, :], cps[:], AF.Silu)

                        # deferred hf1 transposes ride in hf0's PE slots
                        if hf == 0 and 2 in phases and g > 0:
                            for jp in (2 * (g - 1), 2 * (g - 1) + 1):
                                helpers["transpose"](pst_main, 1, jp, eba_hf1.pop(jp))

                        if pend is not None and (1 in phases and 2 in phases):
                            emit_acc(*pend)
                        pend = (g, cvg, k_sb)

                        # hf1 select stream (DVE) spread across hf0's g-loop
                        if hf == 0 and 2 in phases:
                            emit_sel_pair_hf1(2 * g)
                            emit_sel_pair_hf1(2 * g + 1)
                        if g == 1:
                            vsb = value_half(hf)

                    if hf == 0 and 2 in phases:
                        for jp in (6, 7):
                            helpers["transpose"](pst_main, 1, jp, eba_hf1.pop(jp))
                    if pend is not None and (1 in phases and 2 in phases):
                        emit_acc(*pend)

                    # ---- gate ----
                    ra = p_row.tile([4, HT], F32, tag="grow", name=f"ra{hf}")
                    nc.scalar.activation(ra[:], acc_ps[0:4, :], AF.Sqrt,
                                         bias=eps_rms_t[:], scale=1.0 / HID)
                    rb = p_row.tile([4, HT], F32, tag="grow", name=f"rb{hf}")
                    nc.scalar.activation(rb[:], acc_ps[32:36, :], AF.Sqrt,
                                         bias=eps_rms_t[:], scale=1.0 / (HID * ESC * ESC))
                    rm = p_row.tile([4, HT], F32, tag="grow", name=f"rm{hf}")
                    nc.vector.tensor_tensor(out=rm[:], in0=ra[:], in1=rb[:], op=MULT)
                    ri = p_row.tile([4, HT], F32, tag="grow", name=f"ri{hf}")
                    nc.vector.reciprocal(ri[:], rm[:])
                    gp = p_row.tile([4, HT], F32, tag="grow", name=f"gp{hf}")
                    nc.vector.scalar_tensor_tensor(
                        out=gp[:], in0=acc_ps[64:68, :], scalar=1.0 / (32.0 * ESC),
                        in1=ri[:], op0=MULT, op1=MULT)
                    gate_bf = p_row.tile([4, HT], BF, tag="gbf", name=f"gbf{hf}")
                    nc.scalar.activation(gate_bf[:], gp[:], AF.Sigmoid)

                    gbc = p_gb.tile([128, G, HT], BF, tag="gbc")
                    for g in range(G):
                        gb_ps = ps3k.tile([128, HT], F32, tag="kps")
                        nc.tensor.matmul(gb_ps[:], oh_t[:, g, :], gate_bf[:],
                                         start=True, stop=True)
                        nc.scalar.activation(gbc[:, g, :], gb_ps[:], AF.Copy)

                    # ---- ob on DVE (broadcast in0 keeps 2x) + stores ----
                    for dt in range(8):
                        ob = p_ob.tile([128, G, HT], BF, tag="ob")
                        nc.vector.tensor_tensor(
                            out=ob[:],
                            in0=vsb[:, dt, :].rearrange("p (o t) -> p o t", o=1)
                                .to_broadcast((128, G, HT)),
                            in1=gbc[:], op=MULT)
                        nc.sync.dma_start(
                            outT_d[:, dt, :, hf * HT : (hf + 1) * HT]
                                .rearrange("g p t -> p g t"),
                            ob[:])

            if _loop_cm is not None:
                _loop_cm.__exit__(None, None, None)

    nc.finalize()
    return nc


def _host_prep(hidden_states, hash_ids, emb_table, conv_w, sc_norm_w,
               value_w, value_b, key_w, key_b, norm1_w, norm2_w):
    # quad table [NH, NUNITS, 256] bf16, x64, s-interleaved: unit[d*4+s] = row_s[d]
    qt = np.zeros((NH, NUNITS * 4, D), dtype=BF16)
    for j in range(NH):
        n = LIST_OF_N[j]
        qt[j, :n] = (np.asarray(emb_table[OFFSETS[j] : OFFSETS[j] + n],
                                dtype=np.float32) * ESC).astype(BF16)
    # [NH, NUNITS, 4s, 64d] -> [NH, NUNITS, 64d, 4s] -> flatten
    qtab = np.ascontiguousarray(
        qt.reshape(NH, NUNITS, 4, D).transpose(0, 1, 3, 2).reshape(NH, NUNITS, 256))

    vw = (np.asarray(value_w, np.float32) / ESC).astype(BF16)
    vwT = np.empty((8, 128, 8, 128), dtype=BF16)
    for dtile in range(8):
        blk = vw[dtile * 128 : (dtile + 1) * 128, :]
        vwT[dtile] = blk.T.reshape(8, 128, 128).transpose(1, 0, 2)

    kw = (np.asarray(key_w, np.float32) * ESC).astype(F8NP)
    # kw8[g, p_e, o, ep, i, q_o] = kw[g, o*128 + q_o, (2*ep+i)*128 + p_e]
    kw8 = np.ascontiguousarray(
        kw.reshape(G, 8, 128, 4, 2, 128).transpose(0, 5, 1, 3, 4, 2))

    cw = np.asarray(conv_w, np.float32)
    vtap_full = cw[:, 0, :] * np.asarray(sc_norm_w, np.float32).reshape(C)[:, None]  # [C, KS]
    # dg tile layout: [128 i, 8 o, KS j, 128 q]; diag[i, o, j, q] = delta(i,q)*tap
    diag = np.zeros((G, 128, 8, KS, 128), dtype=BF16)
    rr = np.arange(128)
    for g in range(G):
        for o in range(8):
            for j in range(KS):
                diag[g, rr, o, j, rr] = vtap_full[(g * 8 + o) * 128 + rr, j].astype(BF16)

    w12_full = (np.asarray(norm1_w, np.float32) * np.asarray(norm2_w, np.float32)).reshape(C)
    w12p = np.zeros((128, G, 8, 4), dtype=BF16)
    for g in range(G):
        for o in range(8):
            w12p[:, g, o, g] = w12_full[g * HID + o * 128 : g * HID + (o + 1) * 128].astype(BF16)
    onesp = np.zeros((128, G, 4), dtype=BF16)
    for g in range(G):
        onesp[:, g, g] = 1.0
    oh = np.zeros((G, 4, 128), dtype=BF16)
    for g in range(G):
        oh[g, g, :] = 1.0
    ident = np.eye(128, dtype=BF16)
    kb = np.ascontiguousarray(
        (np.asarray(key_b, np.float32) * ESC).reshape(G, 8, 128).transpose(2, 0, 1))
    vb = np.ascontiguousarray(np.asarray(value_b, np.float32).reshape(8, 128).T)

    shared = dict(qtab=qtab, vwT=vwT, kw8=kw8, diag=diag, w12pad=w12p,
                  onespad=onesp, onehot=oh, ident=ident, kb64=kb, vb=vb)

    hs = np.asarray(hidden_states, np.float32).reshape(B, T, C)
    hid = np.asarray(hash_ids, np.int64)
    in_maps = []
    for core in range(N_CORES):
        b, h = core // 2, core % 2
        t0 = h * TOK
        xpad = np.zeros((W, C), dtype=np.float32)
        lo = max(0, t0 - HALO)
        xpad[HALO - (t0 - lo) :] = hs[b, lo : t0 + TOK]
        hT = np.ascontiguousarray(
            xpad.reshape(W, NCH, 128).transpose(2, 1, 0).astype(BF16))

        hashes = hid[b, t0 : t0 + TOK, :]                     # [TOK, NH]
        units = (hashes // 4).astype(np.int16)
        sel = (hashes % 4).astype(np.int64)
        idx16 = np.zeros((128, 2, NH, HT // 16), dtype=np.int16)
        for hf in range(2):
            for j in range(NH):
                u = units[hf * HT : (hf + 1) * HT, j]
                wrapped = u.reshape(HT // 16, 16).T            # [16, HT//16]
                for grp in range(8):
                    idx16[grp * 16 : (grp + 1) * 16, hf, j, :] = wrapped
        # mask[p, hf, j, b, s] = (sel[hf*512 + b*128 + p, j] == s)
        mask = np.zeros((128, 2, NH, 4, 4), dtype=BF16)
        selr = sel.reshape(2, 4, 128, NH)                      # [hf, blk, p, j]
        pp = np.arange(128)
        for hf in range(2):
            for j in range(NH):
                for blk in range(4):
                    mask[pp, hf, j, blk, selr[hf, blk, :, j]] = 1.0

        in_maps.append(dict(shared, hT=hT, idx16=idx16, mask=mask))
    return in_maps


def kernel(**inputs):
    if "nc" not in _CACHE:
        _CACHE["nc"] = _build_bass()
    nc = _CACHE["nc"]
    in_maps = _host_prep(**inputs)
    res = run_bass_kernel_spmd(nc, in_maps, core_ids=list(range(N_CORES)))
    out = np.empty((B, T, G, HID), dtype=np.float32)
    for core in range(N_CORES):
        b, h = core // 2, core % 2
        t0 = h * TOK
        oT = res.results[core]["outT"]  # [G, 8, 128, TOK] bf16
        out[b, t0 : t0 + TOK] = (
            oT.astype(np.float32).reshape(G, HID, TOK).transpose(2, 0, 1))
    return out
